# revision 18
# baseline (speedup 1.0000x reference)
"""Batched KNN (k=16) + mean feature gather on 8 Trainium2 NeuronCores.

Problem: for each of 16384 query points x (3-D), find the 16 nearest
neighbors among 16384 base points y restricted to the same batch id, and
output the mean of their 16-D features.

Strategy (one core per 2048-query shard; batch-sorted ids give per-batch
locality so each core only needs its own y span — no collectives):

1. Scores S[i,j] = 2*x_i.y_j - |y_j|^2 (row-constant -|x|^2 dropped; order
   preserved) via TensorE matmul in bf16 with 3-term split arithmetic
   (f32-accurate), plus a batch-mismatch penalty -65536*(xb-yb)^2 folded in
   as extra contraction slots (exactly cancels for same-batch pairs).
2. Per-row top-16 threshold on VectorE: max8 per 256-wide group, then
   merge the 8*G group candidates with max8/match_replace to get the 16th
   and 17th largest; threshold t = midpoint. The v16+v17 add runs on
   GpSimd (idle otherwise) except for a slice's last block.
3. D = S^T - t via a second matmul (j on partitions) with -t as 3 extra
   bf16-split contraction slots (t transposed via a small DRAM roundtrip,
   or a PE transpose for the final 1-block slice; the -(v16+v17)/2 split
   chain runs on ScalarE+GpSimd, not VectorE). Selection weights evicted
   ~1024 wide (chunk groups) by ScalarE Sign -> +/-1; in the last slice
   alternate groups go to VectorE (idle there) as (D>0)*2 -> {0,2}.
4. Gather: gT[f, i] = sum_j feat[j, f] * w[j, i] on TensorE with feats as
   the stationary operand; out kept TRANSPOSED [16, R]: epilogue
   outT = (gT + colsum)/32 on ScalarE (Identity, per-partition bias),
   contiguous stores issued from ScalarE's DGE ring (the Sync ring has
   ~9us dispatch latency); the host transposes back.

The queries are processed in UNEVEN slices of [4,3,3,3,2,1] row-blocks:
phase C of slice q-1 is emitted interleaved at BLOCK granularity with
phase A of slice q (so the PE queue never serializes on the DVE scan and
stays dense enough to keep the HAM clock gate at 2.4 GHz), and the final
exposed phase C covers only 128 queries.
"""

import os

import numpy as np
import ml_dtypes

import concourse.bass as bass
import concourse.mybir as mybir
from concourse import bacc
from concourse.tile import TileContext
from concourse.bass_utils import run_bass_kernel_spmd

N_CORES = 8
FEAT = 16
PEN = 65536.0
SENTINEL = 16.0  # batch id for padded y rows (real ids are < 8)
NEG_BIG = -3.0e38
Q_BLOCKS = [4, 3, 3, 3, 2, 1]   # 128-row blocks per pipeline slice
NH = len(Q_BLOCKS)

bf16 = ml_dtypes.bfloat16

# contraction slot layout
KS = 3 + 3 + 18  # penalty + y^2 splits + 6 product terms per coordinate
T0 = 32          # threshold rows start here (DMA-to-SBUF needs start % 32 == 0)
KD = T0 + 3      # + 3 threshold split slots (S^T - t matmul only)


def _chunks_per_group(rh, g):
    """Chunks per eviction group: each chunk's D tile sits at a 512-aligned
    PSUM column (matmul outputs must not cross bank boundaries)."""
    return 2


def _act_group(gi, quarter):
    """Which engine evicts the selection weights for chunk-group gi of this
    slice. True -> ScalarE Sign (+/-1, counted in the colsum correction);
    False -> VectorE (D>0)*2 ({0,2}, no correction)."""
    return not (quarter == NH - 1 and gi % 2 == 1)


def _split3(v):
    """3-term bf16 split of a float64 array: v ~ h+m+l, residual ~2^-27 |v|."""
    h = v.astype(bf16)
    r = v - h.astype(np.float64)
    m = r.astype(bf16)
    l = (r - m.astype(np.float64)).astype(bf16)
    return h, m, l


def _build_sides(xc, xbc, yc, ybc):
    """Host prep of the contraction-slot tensors.

    Returns (X [128, R], Y [128, C]) bf16. X rows T0..KD-1 are zeros (filled
    on device with the -t splits); Y rows T0..KD-1 are ones; rows KD..127
    are zeros on both sides.
    """
    R, C = xc.shape[0], yc.shape[0]
    xs, ys = [], []
    xb64 = xbc.astype(np.float64)
    yb64 = ybc.astype(np.float64)
    # batch penalty: accumulates first, exactly cancels when xb == yb
    xs += [-PEN * xb64 * xb64, 2 * PEN * xb64, np.full(R, -PEN)]
    ys += [np.ones(C), yb64, yb64 * yb64]
    # -|y|^2, 3-split
    c = -(yc.astype(np.float64) ** 2).sum(1)
    ch, cm, cl = (t.astype(np.float64) for t in _split3(c))
    xs += [np.ones(R)] * 3
    ys += [ch, cm, cl]
    # products 2*x_k*y_k, 6 split terms per coordinate
    for k in range(3):
        a = 2.0 * xc[:, k].astype(np.float64)
        b = yc[:, k].astype(np.float64)
        ah, am, al = (t.astype(np.float64) for t in _split3(a))
        bh, bm, bl = (t.astype(np.float64) for t in _split3(b))
        for xa, yb_ in [(ah, bh), (ah, bm), (am, bh), (ah, bl), (al, bh), (am, bm)]:
            xs.append(xa)
            ys.append(yb_)
    # zero padding up to T0, then device-filled threshold slots (y side = 1)
    while len(xs) < T0:
        xs.append(np.zeros(R))
        ys.append(np.zeros(C))
    xs += [np.zeros(R)] * 3
    ys += [np.ones(C)] * 3
    Xl = [v.astype(bf16) for v in xs]
    while len(Xl) < 128:
        Xl.append(np.zeros(R, bf16))
    Yl = [v.astype(bf16) for v in ys]
    while len(Yl) < 128:
        Yl.append(np.zeros(C, bf16))
    return np.stack(Xl), np.stack(Yl)


def _build_nc(R, C):
    """Build the Bass graph for one core (SPMD: all cores run this)."""
    rb = R // 128    # query row blocks
    G = C // 128     # candidate chunks (gather/selection granularity)
    GW = 256         # max8 group width
    assert sum(Q_BLOCKS) == rb, (Q_BLOCKS, rb)
    HS = list(Q_BLOCKS)
    RHs = [128 * h for h in HS]
    OFF = [128 * sum(HS[:q]) for q in range(NH)]
    f32 = mybir.dt.float32
    bft = mybir.dt.bfloat16

    nc = bacc.Bacc(name="knn16")
    xk = nc.dram_tensor("xk", [128, R], bft, kind="ExternalInput")
    yk = nc.dram_tensor("yk", [128, C], bft, kind="ExternalInput")
    fe = nc.dram_tensor("fe", [C, FEAT], bft, kind="ExternalInput")
    cs = nc.dram_tensor("cs", [FEAT, NH], f32, kind="ExternalInput")
    td = nc.dram_tensor("td", [3 * R], bft, kind="Internal")
    out = nc.dram_tensor("out", [FEAT, R], f32, kind="ExternalOutput")

    with TileContext(nc) as tc:
        with (
            tc.tile_pool(name="const", bufs=1) as const,
            tc.tile_pool(name="spool", bufs=3, space="PSUM") as spool,
            tc.tile_pool(name="dpool", bufs=2, space="PSUM") as dpool,
            tc.tile_pool(name="gpool", bufs=1, space="PSUM") as gpool,
            tc.tile_pool(name="work", bufs=2) as work,
            tc.tile_pool(name="wpool", bufs=2) as wpool,
        ):
            # xk is split per slice so a slice's t-row readback (write)
            # never serializes against the next slice's score matmuls.
            xk_q = [
                const.tile([128, RHs[q]], bft, name=f"xkq{q}", tag=f"xkq{q}")
                for q in range(NH)
            ]
            yk_sb = const.tile([128, C], bft)
            fe_sb = const.tile([128, G * FEAT], bft)
            cs_sb = const.tile([FEAT, NH], f32)
            zz_sb = const.tile([1, 512], bft)
            id_sb = const.tile([128, 128], bft)
            # [128, 35] staging tile for the last slice's PE-transpose of
            # the threshold splits (cols 32:35 hold them; 0:32 stay zero).
            tq_sb = const.tile([128, KD], bft)

            nc.gpsimd.memset(zz_sb, 0.0)
            nc.gpsimd.memset(tq_sb[:, 0:T0], 0.0)
            from concourse.masks import make_identity

            make_identity(nc, id_sb)

            # input DMAs: first-needed first (xk q0, then yk by 512-col chunk)
            nc.sync.dma_start(out=xk_q[0][:, :], in_=xk[:, 0:RHs[0]])
            for q in range(C // 512):
                nc.sync.dma_start(
                    out=yk_sb[:, q * 512:(q + 1) * 512],
                    in_=yk[:, q * 512:(q + 1) * 512],
                )
            for q in range(1, NH):
                nc.sync.dma_start(
                    out=xk_q[q][:, :],
                    in_=xk[:, OFF[q]:OFF[q] + RHs[q]],
                )
            nc.sync.dma_start(
                out=fe_sb[:, :].rearrange("p (g f) -> p g f", g=G),
                in_=fe[:, :].rearrange("(g p) f -> p g f", p=128),
            )
            nc.sync.dma_start(out=cs_sb[:, :], in_=cs[:, :])

            def zero_bank(zb):
                nc.tensor.matmul(
                    zb,
                    lhsT=zz_sb[0:1, 0:128],
                    rhs=zz_sb[0:1, 0:512],
                    start=True,
                    stop=False,
                    skip_group_check=True,
                )

            # ---------------- phase C emission (chunk-group granular) ------
            gT = [None] * NH
            w_tiles = {}

            def emit_c_group(qr, gi):
                """Emit the D matmuls for chunk-group gi of slice qr plus
                their wide eviction."""
                rh = RHs[qr]
                cpg = _chunks_per_group(rh, G)
                d_ps = dpool.tile([128, 1024], f32, name="d_ps", tag="D")
                for h in range(cpg):
                    jc = gi * cpg + h
                    nc.tensor.matmul(
                        d_ps[:, h * 512:h * 512 + rh],
                        lhsT=yk_sb[0:128, jc * 128:(jc + 1) * 128],
                        rhs=xk_q[qr][0:128, :],
                        start=True,
                        stop=True,
                    )
                # evict 1024 wide when the group is contiguous (rh == 512),
                # else per-chunk (avoids reading the [rh:512) gap columns)
                w_sb = wpool.tile([128, 1024], bft, name="w_sb", tag="W")
                spans = (
                    [(0, 1024)] if rh == 512
                    else [(h * 512, h * 512 + rh) for h in range(cpg)]
                )
                for lo, hi in spans:
                    if _act_group(gi, qr):
                        nc.scalar.activation(
                            out=w_sb[:, lo:hi],
                            in_=d_ps[:, lo:hi],
                            func=mybir.ActivationFunctionType.Sign,
                        )
                    else:
                        nc.vector.tensor_scalar(
                            out=w_sb[:, lo:hi],
                            in0=d_ps[:, lo:hi],
                            scalar1=0.0,
                            scalar2=2.0,
                            op0=mybir.AluOpType.is_gt,
                            op1=mybir.AluOpType.mult,
                        )
                w_tiles[(qr, gi)] = w_sb

            def emit_g_group(qr, gi):
                """Emit the gather matmuls consuming weight group gi."""
                rh = RHs[qr]
                cpg = _chunks_per_group(rh, G)
                w_sb = w_tiles.pop((qr, gi))
                for h in range(cpg):
                    jc = gi * cpg + h
                    nc.tensor.matmul(
                        gT[qr][0:16, 0:rh],
                        lhsT=fe_sb[:, jc * FEAT:(jc + 1) * FEAT],
                        rhs=w_sb[:, h * 512:h * 512 + rh],
                        start=False,
                        stop=(jc == G - 1),
                        skip_group_check=True,
                    )

            def emit_epilogue(qr):
                """outT = (gT + cs)/32 on ScalarE, then contiguous store."""
                rh = RHs[qr]
                outT = work.tile([16, 512], f32, name="outT", tag="outT")
                nc.scalar.activation(
                    out=outT[:, 0:rh],
                    in_=gT[qr][0:16, 0:rh],
                    func=mybir.ActivationFunctionType.Identity,
                    scale=1.0 / 32.0,
                    bias=cs_sb[:, qr:qr + 1],
                )
                # store via ScalarE's DGE ring: the Sync-issued DRAM store
                # lands on a ring with ~9us dispatch latency.
                nc.scalar.dma_start(
                    out=out[:, OFF[qr]:OFF[qr] + rh],
                    in_=outT[:, 0:rh],
                )

            # per-slice plan: which C-groups of slice qr-1 run in each
            # block-slot of slice qr (spread as evenly as possible)
            def group_plan(n_groups, n_slots):
                return [
                    range(i * n_groups // n_slots, (i + 1) * n_groups // n_slots)
                    for i in range(n_slots)
                ]

            # ---------------- main loop -----------------------------------
            for qr in range(NH):
                hs, rh, off = HS[qr], RHs[qr], OFF[qr]
                # phase A "blocks" are strided column sets of this slice:
                # block b covers xk columns off + p*hs + b (p = 0..127),
                # which makes the t scatter's last dim contiguous.
                xk_str = xk_q[qr][0:128, :].rearrange("k (p b) -> k b p", b=hs)
                t_all = work.tile([128, hs], f32, name="t_all", tag="tall")

                if qr > 0:
                    prev_rh = RHs[qr - 1]
                    prev_ng = G // _chunks_per_group(prev_rh, G)
                    plan = group_plan(prev_ng, hs)
                    gT[qr - 1] = gpool.tile([128, 512], f32, name="gT", tag="gT")
                    zero_bank(gT[qr - 1])

                for bi in range(hs):
                    # ---- phase A: scores + per-row top-16/17 values ----
                    cand = work.tile([128, (C // GW) * 8], f32, name="cand", tag="cand")
                    for q in range(C // 512):
                        s_ps = spool.tile([128, 512], f32, name="s_ps", tag="S")
                        nc.tensor.matmul(
                            s_ps,
                            lhsT=xk_str[:, bi, :],
                            rhs=yk_sb[0:128, q * 512:(q + 1) * 512],
                            start=True,
                            stop=True,
                        )
                        for g in range(512 // GW):
                            gi = q * (512 // GW) + g
                            nc.vector.max(
                                out=cand[:, gi * 8:(gi + 1) * 8],
                                in_=s_ps[:, g * GW:(g + 1) * GW],
                            )
                    # ---- phase C of the previous slice, interleaved ----
                    if qr > 0:
                        for gi in plan[bi]:
                            emit_c_group(qr - 1, gi)
                            if gi > 0:
                                emit_g_group(qr - 1, gi - 1)
                    # merge: 16th + 17th largest of the group winners
                    m1 = work.tile([128, 8], f32, name="m1", tag="m1")
                    nc.vector.max(out=m1, in_=cand)
                    cand2 = work.tile([128, (C // GW) * 8], f32, name="cand2", tag="cand2")
                    nc.vector.match_replace(
                        out=cand2, in_to_replace=m1, in_values=cand,
                        imm_value=NEG_BIG,
                    )
                    m2 = work.tile([128, 8], f32, name="m2", tag="m2")
                    nc.vector.max(out=m2, in_=cand2)
                    cand3 = work.tile([128, (C // GW) * 8], f32, name="cand3", tag="cand3")
                    nc.vector.match_replace(
                        out=cand3, in_to_replace=m2, in_values=cand2,
                        imm_value=NEG_BIG,
                    )
                    # v16+v17 off the DVE queue (GpSimd is idle), except
                    # the slice's last block (tsplit waits on it).
                    eng = nc.gpsimd if bi < hs - 1 else nc.vector
                    v17 = work.tile([128, 1], f32, name="v17", tag="v17")
                    nc.vector.tensor_reduce(
                        out=v17, in_=cand3, axis=mybir.AxisListType.X,
                        op=mybir.AluOpType.max,
                    )
                    eng.tensor_add(
                        out=t_all[:, bi:bi + 1], in0=m2[:, 7:8], in1=v17,
                    )

                # batched threshold split: tneg = -(v16+v17)/2 as 3 bf16
                # terms, on ScalarE (casts) + GpSimd (residuals).
                last = qr == NH - 1
                if last:
                    # write splits into tq_sb cols 32:35 for the PE transpose
                    tsplit = tq_sb[:, T0:KD].rearrange("p (s b) -> p s b", b=hs)
                else:
                    tsp_t = work.tile([128, 3, hs], bft, name="tsp", tag="tsplit")
                    tsplit = tsp_t[:, :, :]
                th_ = tsplit[:, 0, :]
                tm_ = tsplit[:, 1, :]
                tl_ = tsplit[:, 2, :]
                r1 = work.tile([128, hs], f32, name="r1", tag="r1")
                r2 = work.tile([128, hs], f32, name="r2", tag="r2")
                nh_t = work.tile([128, hs], f32, name="nh_t", tag="nht")
                nc.scalar.activation(
                    out=th_, in_=t_all[:, :],
                    func=mybir.ActivationFunctionType.Copy, scale=-0.5,
                )
                nc.scalar.activation(
                    out=nh_t, in_=t_all[:, :],
                    func=mybir.ActivationFunctionType.Copy, scale=-0.5,
                )
                nc.gpsimd.tensor_sub(out=r1, in0=nh_t, in1=th_)
                nc.scalar.activation(
                    out=tm_, in_=r1,
                    func=mybir.ActivationFunctionType.Copy,
                )
                nc.gpsimd.tensor_sub(out=r2, in0=r1, in1=tm_)
                nc.scalar.activation(
                    out=tl_, in_=r2,
                    func=mybir.ActivationFunctionType.Copy,
                )

                # ---- phase B: transpose tneg into xk_q rows T0:KD ----
                if last:
                    # PE transpose (by now phase A is done, spool is free):
                    # tq_sb [128, 35] -> tr_ps [35, 128]; rows 32:35 are the
                    # splits, at the right partitions for an ACT copy.
                    tr_ps = spool.tile([128, 1024], bft, name="tr_ps", tag="S")
                    nc.tensor.matmul(
                        tr_ps[0:KD, 0:128],
                        lhsT=tq_sb[:, :],
                        rhs=id_sb[:, :],
                        is_transpose=True,
                        start=True,
                        stop=True,
                        skip_group_check=True,
                    )
                    nc.scalar.activation(
                        out=xk_q[qr][T0:KD, :],
                        in_=tr_ps[T0:KD, 0:rh],
                        func=mybir.ActivationFunctionType.Copy,
                    )
                else:
                    # DRAM roundtrip: td flat layout addr = s*R + off + p*hs + b
                    with nc.allow_non_contiguous_dma("t transpose scatter"):
                        nc.sync.dma_start(
                            out=bass.AP(td, off, [[hs, 128], [R, 3], [1, hs]]),
                            in_=tsplit,
                        )
                    nc.sync.dma_start(
                        out=xk_q[qr][T0:KD, :],
                        in_=bass.AP(td, off, [[R, 3], [1, rh]]),
                    )

                if qr > 0:
                    emit_g_group(qr - 1, prev_ng - 1)
                    emit_epilogue(qr - 1)

            # ---- tail: phase C of the last slice ----
            qr = NH - 1
            ng = G // _chunks_per_group(RHs[qr], G)
            gT[qr] = gpool.tile([128, 512], f32, name="gT", tag="gT")
            zero_bank(gT[qr])
            for gi in range(ng):
                emit_c_group(qr, gi)
                if gi > 0:
                    emit_g_group(qr, gi - 1)
            emit_g_group(qr, ng - 1)
            emit_epilogue(qr)
    nc.finalize()
    return nc


_NC_CACHE = {}


def _get_nc(R, C):
    key = (R, C)
    if key not in _NC_CACHE:
        _NC_CACHE[key] = _build_nc(R, C)
    return _NC_CACHE[key]


def kernel(x, y, y_atomflex, x_batch, y_batch):
    x = np.ascontiguousarray(np.asarray(x, dtype=np.float32))
    y = np.ascontiguousarray(np.asarray(y, dtype=np.float32))
    feats = np.ascontiguousarray(np.asarray(y_atomflex, dtype=np.float32))
    xb = np.asarray(x_batch).astype(np.int64)
    yb = np.asarray(y_batch).astype(np.int64)

    N = x.shape[0]
    R = N // N_CORES

    # per-core y spans (batch ids are sorted)
    spans = []
    for c in range(N_CORES):
        blo, bhi = xb[c * R], xb[(c + 1) * R - 1]
        s = int(np.searchsorted(yb, blo, "left"))
        e = int(np.searchsorted(yb, bhi, "right"))
        spans.append((s, e))
    C = max(1024, -(-max(e - s for s, e in spans) // 1024) * 1024)
    G = C // 128

    in_maps = []
    for c in range(N_CORES):
        s, e = spans[c]
        n = e - s
        yc = np.zeros((C, 3), np.float32)
        yc[:n] = y[s:e]
        ybc = np.full(C, SENTINEL)
        ybc[:n] = yb[s:e]
        fec = np.zeros((C, FEAT), np.float32)
        fec[:n] = feats[s:e]
        fe_bf = fec.astype(bf16)
        X, Y = _build_sides(x[c * R:(c + 1) * R], xb[c * R:(c + 1) * R], yc, ybc)
        # per-slice colsum over the Sign (+/-1) chunk groups, pre-divided
        # by 32 (the ScalarE epilogue computes gT/32 + cs)
        csq = np.zeros((FEAT, NH), np.float64)
        for qr in range(NH):
            rh = 128 * Q_BLOCKS[qr]
            cpg = _chunks_per_group(rh, G)
            mask = np.zeros(C, np.float64)
            for gi in range(G // cpg):
                if _act_group(gi, qr):
                    mask[gi * cpg * 128:(gi + 1) * cpg * 128] = 1.0
            csq[:, qr] = (fe_bf.astype(np.float64) * mask[:, None]).sum(0) / 32.0
        in_maps.append(
            {
                "xk": np.ascontiguousarray(X),
                "yk": np.ascontiguousarray(Y),
                "fe": np.ascontiguousarray(fe_bf),
                "cs": np.ascontiguousarray(csq.astype(np.float32)),
            }
        )

    nc = _get_nc(R, C)
    trace = bool(int(os.environ.get("KNN_TRACE", "0")))
    res = run_bass_kernel_spmd(
        nc, in_maps, core_ids=list(range(N_CORES)), trace=trace
    )
    if trace and res.exec_time_ns is not None:
        print(f"HW exec time: {res.exec_time_ns} ns")
        if res.instructions_and_trace is not None:
            print(f"trace: {res.instructions_and_trace[1]}")

    out = np.concatenate([r["out"].T for r in res.results], axis=0)
    return np.ascontiguousarray(out.astype(np.float32))


if __name__ == "__main__":
    # smoke test against the local reference
    import reference

    inputs = {k: np.asarray(v) for k, v in reference.setup_inputs().items()}
    expected = np.asarray(reference.reference(**inputs))
    actual = kernel(**inputs)
    err = np.linalg.norm(actual - expected) / np.linalg.norm(expected)
    print(f"Relative error: {err:.6f}")


# revision 20
# speedup vs baseline: 1.2394x; 1.2394x over previous
"""Batched KNN (k=16) + mean feature gather on 8 Trainium2 NeuronCores.

Problem: for each of 16384 query points x (3-D), find the 16 nearest
neighbors among 16384 base points y restricted to the same batch id, and
output the mean of their 16-D features.

Strategy (one core per 2048-query shard; batch-sorted ids give per-batch
locality so each core only needs its own y span — no collectives):

1. Scores S[i,j] = 2*x_i.y_j - |y_j|^2 (row-constant -|x|^2 dropped; order
   preserved) via TensorE matmul in bf16 with 3-term split arithmetic
   (f32-accurate), plus a batch-mismatch penalty -65536*(xb-yb)^2 folded in
   as extra contraction slots (exactly cancels for same-batch pairs).
2. Per-row top-16 threshold on VectorE: max8 per 256-wide group, then
   merge the 8*G group candidates with max8/match_replace to get the 16th
   and 17th largest; threshold t = midpoint. The v16+v17 add runs on
   GpSimd (idle otherwise) except for a slice's last block.
3. D = S^T - t via a second matmul (j on partitions) with -t as 3 extra
   bf16-split contraction slots (t transposed via a small DRAM roundtrip,
   or a PE transpose for the final 1-block slice; the -(v16+v17)/2 split
   chain runs on ScalarE+GpSimd, not VectorE). Selection weights evicted
   ~1024 wide (chunk groups) by ScalarE Sign -> +/-1; in the last slice
   alternate groups go to VectorE (idle there) as (D>0)*2 -> {0,2}.
4. Gather: gT[f, i] = sum_j feat[j, f] * w[j, i] on TensorE with feats as
   the stationary operand; out kept TRANSPOSED [16, R]: epilogue
   outT = (gT + colsum)/32 on ScalarE (Identity, per-partition bias),
   contiguous stores issued from ScalarE's DGE ring (the Sync ring has
   ~9us dispatch latency); the host transposes back.

The queries are processed in UNEVEN slices of [4,3,3,3,2,1] row-blocks:
phase C of slice q-1 is emitted interleaved at BLOCK granularity with
phase A of slice q (so the PE queue never serializes on the DVE scan and
stays dense enough to keep the HAM clock gate at 2.4 GHz), and the final
exposed phase C covers only 128 queries.
"""

import os

import numpy as np
import ml_dtypes

import concourse.bass as bass
import concourse.mybir as mybir
from concourse import bacc
from concourse.tile import TileContext
from concourse.bass_utils import run_bass_kernel_spmd

N_CORES = 8
FEAT = 16
PEN = 65536.0
SENTINEL = 16.0  # batch id for padded y rows (real ids are < 8)
NEG_BIG = -3.0e38
Q_BLOCKS = [4, 4, 4, 4]   # 128-row blocks per pipeline slice
NH = len(Q_BLOCKS)

bf16 = ml_dtypes.bfloat16

# contraction slot layout
KS = 3 + 3 + 18  # penalty + y^2 splits + 6 product terms per coordinate
T0 = 32          # threshold rows start here (DMA-to-SBUF needs start % 32 == 0)
KD = T0 + 3      # + 3 threshold split slots (S^T - t matmul only)


def _chunks_per_group(rh, g):
    """Chunks per eviction group: each chunk's D tile sits at a 512-aligned
    PSUM column (matmul outputs must not cross bank boundaries)."""
    return 2


def _act_group(gi, quarter):
    """Which engine evicts the selection weights for chunk-group gi of this
    slice. True -> ScalarE Sign (+/-1, counted in the colsum correction);
    False -> VectorE (D>0)*2 ({0,2}, no correction)."""
    return not (quarter == NH - 1 and gi % 2 == 1)


def _split3(v):
    """3-term bf16 split of a float64 array: v ~ h+m+l, residual ~2^-27 |v|."""
    h = v.astype(bf16)
    r = v - h.astype(np.float64)
    m = r.astype(bf16)
    l = (r - m.astype(np.float64)).astype(bf16)
    return h, m, l


def _build_sides(xc, xbc, yc, ybc):
    """Host prep of the contraction-slot tensors.

    Returns (X [128, R], Y [128, C]) bf16. X rows T0..KD-1 are zeros (filled
    on device with the -t splits); Y rows T0..KD-1 are ones; rows KD..127
    are zeros on both sides.
    """
    R, C = xc.shape[0], yc.shape[0]
    xs, ys = [], []
    xb64 = xbc.astype(np.float64)
    yb64 = ybc.astype(np.float64)
    # batch penalty: accumulates first, exactly cancels when xb == yb
    xs += [-PEN * xb64 * xb64, 2 * PEN * xb64, np.full(R, -PEN)]
    ys += [np.ones(C), yb64, yb64 * yb64]
    # -|y|^2, 3-split
    c = -(yc.astype(np.float64) ** 2).sum(1)
    ch, cm, cl = (t.astype(np.float64) for t in _split3(c))
    xs += [np.ones(R)] * 3
    ys += [ch, cm, cl]
    # products 2*x_k*y_k, 6 split terms per coordinate
    for k in range(3):
        a = 2.0 * xc[:, k].astype(np.float64)
        b = yc[:, k].astype(np.float64)
        ah, am, al = (t.astype(np.float64) for t in _split3(a))
        bh, bm, bl = (t.astype(np.float64) for t in _split3(b))
        for xa, yb_ in [(ah, bh), (ah, bm), (am, bh), (ah, bl), (al, bh), (am, bm)]:
            xs.append(xa)
            ys.append(yb_)
    # zero padding up to T0, then device-filled threshold slots (y side = 1)
    while len(xs) < T0:
        xs.append(np.zeros(R))
        ys.append(np.zeros(C))
    xs += [np.zeros(R)] * 3
    ys += [np.ones(C)] * 3
    Xl = [v.astype(bf16) for v in xs]
    while len(Xl) < 128:
        Xl.append(np.zeros(R, bf16))
    Yl = [v.astype(bf16) for v in ys]
    while len(Yl) < 128:
        Yl.append(np.zeros(C, bf16))
    return np.stack(Xl), np.stack(Yl)


def _build_nc(R, C):
    """Build the Bass graph for one core (SPMD: all cores run this)."""
    rb = R // 128    # query row blocks
    G = C // 128     # candidate chunks (gather/selection granularity)
    GW = 256         # max8 group width
    assert sum(Q_BLOCKS) == rb, (Q_BLOCKS, rb)
    HS = list(Q_BLOCKS)
    RHs = [128 * h for h in HS]
    OFF = [128 * sum(HS[:q]) for q in range(NH)]
    f32 = mybir.dt.float32
    bft = mybir.dt.bfloat16

    nc = bacc.Bacc(name="knn16")
    xk = nc.dram_tensor("xk", [128, R], bft, kind="ExternalInput")
    yk = nc.dram_tensor("yk", [128, C], bft, kind="ExternalInput")
    fe = nc.dram_tensor("fe", [C, FEAT], bft, kind="ExternalInput")
    cs = nc.dram_tensor("cs", [FEAT, NH], f32, kind="ExternalInput")
    td = nc.dram_tensor("td", [3 * R], bft, kind="Internal")
    out = nc.dram_tensor("out", [FEAT, R], f32, kind="ExternalOutput")

    with TileContext(nc) as tc:
        with (
            tc.tile_pool(name="const", bufs=1) as const,
            tc.tile_pool(name="spool", bufs=3, space="PSUM") as spool,
            tc.tile_pool(name="dpool", bufs=2, space="PSUM") as dpool,
            tc.tile_pool(name="gpool", bufs=1, space="PSUM") as gpool,
            tc.tile_pool(name="work", bufs=2) as work,
            tc.tile_pool(name="wpool", bufs=2) as wpool,
        ):
            # xk is split per slice so a slice's t-row readback (write)
            # never serializes against the next slice's score matmuls.
            xk_q = [
                const.tile([128, RHs[q]], bft, name=f"xkq{q}", tag=f"xkq{q}")
                for q in range(NH)
            ]
            yk_sb = const.tile([128, C], bft)
            fe_sb = const.tile([128, G * FEAT], bft)
            cs_sb = const.tile([FEAT, NH], f32)
            zz_sb = const.tile([1, 512], bft)
            id_sb = const.tile([128, 128], bft)
            # [128, 35] staging tile for the last slice's PE-transpose of
            # the threshold splits (cols 32:35 hold them; 0:32 stay zero).
            tq_sb = const.tile([128, KD], bft)

            nc.gpsimd.memset(zz_sb, 0.0)
            nc.gpsimd.memset(tq_sb[:, 0:T0], 0.0)
            from concourse.masks import make_identity

            make_identity(nc, id_sb)

            # input DMAs: first-needed first (xk q0, then yk by 512-col chunk)
            nc.sync.dma_start(out=xk_q[0][:, :], in_=xk[:, 0:RHs[0]])
            for q in range(C // 512):
                nc.sync.dma_start(
                    out=yk_sb[:, q * 512:(q + 1) * 512],
                    in_=yk[:, q * 512:(q + 1) * 512],
                )
            for q in range(1, NH):
                nc.sync.dma_start(
                    out=xk_q[q][:, :],
                    in_=xk[:, OFF[q]:OFF[q] + RHs[q]],
                )
            nc.sync.dma_start(
                out=fe_sb[:, :].rearrange("p (g f) -> p g f", g=G),
                in_=fe[:, :].rearrange("(g p) f -> p g f", p=128),
            )
            nc.sync.dma_start(out=cs_sb[:, :], in_=cs[:, :])

            def zero_bank(zb):
                nc.tensor.matmul(
                    zb,
                    lhsT=zz_sb[0:1, 0:128],
                    rhs=zz_sb[0:1, 0:512],
                    start=True,
                    stop=False,
                    skip_group_check=True,
                )

            # ---------------- phase C emission (chunk-group granular) ------
            gT = [None] * NH
            w_tiles = {}

            def emit_c_group(qr, gi):
                """Emit the D matmuls for chunk-group gi of slice qr plus
                their wide eviction."""
                rh = RHs[qr]
                cpg = _chunks_per_group(rh, G)
                d_ps = dpool.tile([128, 1024], f32, name="d_ps", tag="D")
                for h in range(cpg):
                    jc = gi * cpg + h
                    nc.tensor.matmul(
                        d_ps[:, h * 512:h * 512 + rh],
                        lhsT=yk_sb[0:128, jc * 128:(jc + 1) * 128],
                        rhs=xk_q[qr][0:128, :],
                        start=True,
                        stop=True,
                    )
                # evict 1024 wide when the group is contiguous (rh == 512),
                # else per-chunk (avoids reading the [rh:512) gap columns)
                w_sb = wpool.tile([128, 1024], bft, name="w_sb", tag="W")
                spans = (
                    [(0, 1024)] if rh == 512
                    else [(h * 512, h * 512 + rh) for h in range(cpg)]
                )
                for lo, hi in spans:
                    if _act_group(gi, qr):
                        nc.scalar.activation(
                            out=w_sb[:, lo:hi],
                            in_=d_ps[:, lo:hi],
                            func=mybir.ActivationFunctionType.Sign,
                        )
                    else:
                        nc.vector.tensor_scalar(
                            out=w_sb[:, lo:hi],
                            in0=d_ps[:, lo:hi],
                            scalar1=0.0,
                            scalar2=2.0,
                            op0=mybir.AluOpType.is_gt,
                            op1=mybir.AluOpType.mult,
                        )
                w_tiles[(qr, gi)] = w_sb

            def emit_g_group(qr, gi):
                """Emit the gather matmuls consuming weight group gi."""
                rh = RHs[qr]
                cpg = _chunks_per_group(rh, G)
                w_sb = w_tiles.pop((qr, gi))
                for h in range(cpg):
                    jc = gi * cpg + h
                    nc.tensor.matmul(
                        gT[qr][0:16, 0:rh],
                        lhsT=fe_sb[:, jc * FEAT:(jc + 1) * FEAT],
                        rhs=w_sb[:, h * 512:h * 512 + rh],
                        start=False,
                        stop=(jc == G - 1),
                        skip_group_check=True,
                    )

            def emit_epilogue(qr):
                """outT = (gT + cs)/32 on ScalarE, then contiguous store."""
                rh = RHs[qr]
                outT = work.tile([16, 512], f32, name="outT", tag="outT")
                nc.scalar.activation(
                    out=outT[:, 0:rh],
                    in_=gT[qr][0:16, 0:rh],
                    func=mybir.ActivationFunctionType.Identity,
                    scale=1.0 / 32.0,
                    bias=cs_sb[:, qr:qr + 1],
                )
                # store via ScalarE's DGE ring: the Sync-issued DRAM store
                # lands on a ring with ~9us dispatch latency.
                nc.scalar.dma_start(
                    out=out[:, OFF[qr]:OFF[qr] + rh],
                    in_=outT[:, 0:rh],
                )

            # per-slice plan: which C-groups of slice qr-1 run in each
            # block-slot of slice qr (spread as evenly as possible)
            def group_plan(n_groups, n_slots):
                return [
                    range(i * n_groups // n_slots, (i + 1) * n_groups // n_slots)
                    for i in range(n_slots)
                ]

            # ---------------- main loop -----------------------------------
            for qr in range(NH):
                hs, rh, off = HS[qr], RHs[qr], OFF[qr]
                # phase A "blocks" are strided column sets of this slice:
                # block b covers xk columns off + p*hs + b (p = 0..127),
                # which makes the t scatter's last dim contiguous.
                xk_str = xk_q[qr][0:128, :].rearrange("k (p b) -> k b p", b=hs)
                t_all = work.tile([128, hs], f32, name="t_all", tag="tall")

                if qr > 0:
                    prev_rh = RHs[qr - 1]
                    prev_ng = G // _chunks_per_group(prev_rh, G)
                    plan = group_plan(prev_ng, hs)
                    gT[qr - 1] = gpool.tile([128, 512], f32, name="gT", tag="gT")
                    zero_bank(gT[qr - 1])

                for bi in range(hs):
                    # ---- phase A: scores + per-row top-16/17 values ----
                    cand = work.tile([128, (C // GW) * 8], f32, name="cand", tag="cand")
                    for q in range(C // 512):
                        s_ps = spool.tile([128, 512], f32, name="s_ps", tag="S")
                        nc.tensor.matmul(
                            s_ps,
                            lhsT=xk_str[:, bi, :],
                            rhs=yk_sb[0:128, q * 512:(q + 1) * 512],
                            start=True,
                            stop=True,
                        )
                        for g in range(512 // GW):
                            gi = q * (512 // GW) + g
                            nc.vector.max(
                                out=cand[:, gi * 8:(gi + 1) * 8],
                                in_=s_ps[:, g * GW:(g + 1) * GW],
                            )
                    # ---- phase C of the previous slice, interleaved ----
                    if qr > 0:
                        for gi in plan[bi]:
                            emit_c_group(qr - 1, gi)
                            if gi > 0:
                                emit_g_group(qr - 1, gi - 1)
                    # merge: 16th + 17th largest of the group winners
                    m1 = work.tile([128, 8], f32, name="m1", tag="m1")
                    nc.vector.max(out=m1, in_=cand)
                    cand2 = work.tile([128, (C // GW) * 8], f32, name="cand2", tag="cand2")
                    nc.vector.match_replace(
                        out=cand2, in_to_replace=m1, in_values=cand,
                        imm_value=NEG_BIG,
                    )
                    m2 = work.tile([128, 8], f32, name="m2", tag="m2")
                    nc.vector.max(out=m2, in_=cand2)
                    cand3 = work.tile([128, (C // GW) * 8], f32, name="cand3", tag="cand3")
                    nc.vector.match_replace(
                        out=cand3, in_to_replace=m2, in_values=cand2,
                        imm_value=NEG_BIG,
                    )
                    # v16+v17 off the DVE queue (GpSimd is idle), except
                    # the slice's last block (tsplit waits on it).
                    eng = nc.gpsimd if bi < hs - 1 else nc.vector
                    v17 = work.tile([128, 1], f32, name="v17", tag="v17")
                    nc.vector.tensor_reduce(
                        out=v17, in_=cand3, axis=mybir.AxisListType.X,
                        op=mybir.AluOpType.max,
                    )
                    eng.tensor_add(
                        out=t_all[:, bi:bi + 1], in0=m2[:, 7:8], in1=v17,
                    )

                # batched threshold split: tneg = -(v16+v17)/2 as 3 bf16
                # terms, on ScalarE (casts) + GpSimd (residuals).
                last = qr == NH - 1 and hs == 1
                if last:
                    # write splits into tq_sb cols 32:35 for the PE transpose
                    tsplit = tq_sb[:, T0:KD].rearrange("p (s b) -> p s b", b=hs)
                else:
                    tsp_t = work.tile([128, 3, hs], bft, name="tsp", tag="tsplit")
                    tsplit = tsp_t[:, :, :]
                th_ = tsplit[:, 0, :]
                tm_ = tsplit[:, 1, :]
                tl_ = tsplit[:, 2, :]
                r1 = work.tile([128, hs], f32, name="r1", tag="r1")
                r2 = work.tile([128, hs], f32, name="r2", tag="r2")
                nh_t = work.tile([128, hs], f32, name="nh_t", tag="nht")
                nc.scalar.activation(
                    out=th_, in_=t_all[:, :],
                    func=mybir.ActivationFunctionType.Copy, scale=-0.5,
                )
                nc.scalar.activation(
                    out=nh_t, in_=t_all[:, :],
                    func=mybir.ActivationFunctionType.Copy, scale=-0.5,
                )
                nc.gpsimd.tensor_sub(out=r1, in0=nh_t, in1=th_)
                nc.scalar.activation(
                    out=tm_, in_=r1,
                    func=mybir.ActivationFunctionType.Copy,
                )
                nc.gpsimd.tensor_sub(out=r2, in0=r1, in1=tm_)
                nc.scalar.activation(
                    out=tl_, in_=r2,
                    func=mybir.ActivationFunctionType.Copy,
                )

                # ---- phase B: transpose tneg into xk_q rows T0:KD ----
                if last:
                    # PE transpose (by now phase A is done, spool is free):
                    # tq_sb [128, 35] -> tr_ps [35, 128]; rows 32:35 are the
                    # splits, at the right partitions for an ACT copy.
                    tr_ps = spool.tile([128, 1024], bft, name="tr_ps", tag="S")
                    nc.tensor.matmul(
                        tr_ps[0:KD, 0:128],
                        lhsT=tq_sb[:, :],
                        rhs=id_sb[:, :],
                        is_transpose=True,
                        start=True,
                        stop=True,
                        skip_group_check=True,
                    )
                    nc.scalar.activation(
                        out=xk_q[qr][T0:KD, :],
                        in_=tr_ps[T0:KD, 0:rh],
                        func=mybir.ActivationFunctionType.Copy,
                    )
                else:
                    # DRAM roundtrip: td flat layout addr = s*R + off + p*hs + b
                    with nc.allow_non_contiguous_dma("t transpose scatter"):
                        nc.sync.dma_start(
                            out=bass.AP(td, off, [[hs, 128], [R, 3], [1, hs]]),
                            in_=tsplit,
                        )
                    nc.sync.dma_start(
                        out=xk_q[qr][T0:KD, :],
                        in_=bass.AP(td, off, [[R, 3], [1, rh]]),
                    )

                if qr > 0:
                    emit_g_group(qr - 1, prev_ng - 1)
                    emit_epilogue(qr - 1)

            # ---- tail: phase C of the last slice ----
            qr = NH - 1
            ng = G // _chunks_per_group(RHs[qr], G)
            gT[qr] = gpool.tile([128, 512], f32, name="gT", tag="gT")
            zero_bank(gT[qr])
            for gi in range(ng):
                emit_c_group(qr, gi)
                if gi > 0:
                    emit_g_group(qr, gi - 1)
            emit_g_group(qr, ng - 1)
            emit_epilogue(qr)
    nc.finalize()
    return nc


_NC_CACHE = {}


def _get_nc(R, C):
    key = (R, C)
    if key not in _NC_CACHE:
        _NC_CACHE[key] = _build_nc(R, C)
    return _NC_CACHE[key]


def kernel(x, y, y_atomflex, x_batch, y_batch):
    x = np.ascontiguousarray(np.asarray(x, dtype=np.float32))
    y = np.ascontiguousarray(np.asarray(y, dtype=np.float32))
    feats = np.ascontiguousarray(np.asarray(y_atomflex, dtype=np.float32))
    xb = np.asarray(x_batch).astype(np.int64)
    yb = np.asarray(y_batch).astype(np.int64)

    N = x.shape[0]
    R = N // N_CORES

    # per-core y spans (batch ids are sorted)
    spans = []
    for c in range(N_CORES):
        blo, bhi = xb[c * R], xb[(c + 1) * R - 1]
        s = int(np.searchsorted(yb, blo, "left"))
        e = int(np.searchsorted(yb, bhi, "right"))
        spans.append((s, e))
    C = max(1024, -(-max(e - s for s, e in spans) // 1024) * 1024)
    G = C // 128

    in_maps = []
    for c in range(N_CORES):
        s, e = spans[c]
        n = e - s
        yc = np.zeros((C, 3), np.float32)
        yc[:n] = y[s:e]
        ybc = np.full(C, SENTINEL)
        ybc[:n] = yb[s:e]
        fec = np.zeros((C, FEAT), np.float32)
        fec[:n] = feats[s:e]
        fe_bf = fec.astype(bf16)
        X, Y = _build_sides(x[c * R:(c + 1) * R], xb[c * R:(c + 1) * R], yc, ybc)
        # per-slice colsum over the Sign (+/-1) chunk groups, pre-divided
        # by 32 (the ScalarE epilogue computes gT/32 + cs)
        csq = np.zeros((FEAT, NH), np.float64)
        for qr in range(NH):
            rh = 128 * Q_BLOCKS[qr]
            cpg = _chunks_per_group(rh, G)
            mask = np.zeros(C, np.float64)
            for gi in range(G // cpg):
                if _act_group(gi, qr):
                    mask[gi * cpg * 128:(gi + 1) * cpg * 128] = 1.0
            csq[:, qr] = (fe_bf.astype(np.float64) * mask[:, None]).sum(0) / 32.0
        in_maps.append(
            {
                "xk": np.ascontiguousarray(X),
                "yk": np.ascontiguousarray(Y),
                "fe": np.ascontiguousarray(fe_bf),
                "cs": np.ascontiguousarray(csq.astype(np.float32)),
            }
        )

    nc = _get_nc(R, C)
    trace = bool(int(os.environ.get("KNN_TRACE", "0")))
    res = run_bass_kernel_spmd(
        nc, in_maps, core_ids=list(range(N_CORES)), trace=trace
    )
    if trace and res.exec_time_ns is not None:
        print(f"HW exec time: {res.exec_time_ns} ns")
        if res.instructions_and_trace is not None:
            print(f"trace: {res.instructions_and_trace[1]}")

    out = np.concatenate([r["out"].T for r in res.results], axis=0)
    return np.ascontiguousarray(out.astype(np.float32))


if __name__ == "__main__":
    # smoke test against the local reference
    import reference

    inputs = {k: np.asarray(v) for k, v in reference.setup_inputs().items()}
    expected = np.asarray(reference.reference(**inputs))
    actual = kernel(**inputs)
    err = np.linalg.norm(actual - expected) / np.linalg.norm(expected)
    print(f"Relative error: {err:.6f}")
